# revision 1
# baseline (speedup 1.0000x reference)
"""Trainium2 Bass kernel for nn_MultiHeadDecoder (sparse neighbour compat + MLP + softmax).

Strategy (data-parallel over batch, 8 batches per core):
 - Host: decompose the `rec` permutation into cycles and lay nodes out in tour
   order (with per-cycle pad columns) so predecessor / succ^2 lookups become
   free-dim column shifts on-chip.  The per-core shard is shipped pre-gathered
   and feature-major: hemt[b] = h_em[b][order].T  ([128, PEXT]).
 - Algebra folding (host, float64): the reference's per-head Q/K projections of
   h = h_em @ Wn.T + g-proj collapse into one bilinear form per head:
       compat[pos p] = (A_h[:,p-1]+c_h).F[:,p] + (A_h[:,p]+c_h).E[:,p+2]  (+s)
   where A_h = Mt_h^T E, Mt_h = Wn^T Wq_h Wk_h^T Wn, F = E - shift2(E), and the
   per-batch scalars c_h (from the graph-max projection) ride the DVE op while
   s folds into the first MLP bias.  This halves matmul work vs explicit Q/K.
 - Device: one 128x128 matmul per head (PSUM), fused (A+c)*F products on DVE
   straight out of PSUM (scalar_tensor_tensor), per-position dot reduction as a
   TensorE column-sum matmul (lhsT=ones) writing compat feature-major
   [4 heads, positions], GPSIMD ap_gather to join pickup/delivery tour
   positions into node order, then the 12->32->32->1 MLP + tanh + softmax.
"""
import os
import sys
from contextlib import ExitStack

import numpy as np

for _p in ("/opt/trn_rl_repo", "/root/.axon_site/_ro/trn_rl_repo"):
    if os.path.isdir(_p) and _p not in sys.path:
        sys.path.insert(0, _p)

import concourse.bacc as bacc
import concourse.bass as bass
import concourse.mybir as mybir
import concourse.tile as tile
from concourse.bass_utils import run_bass_kernel_spmd
from concourse.library_config import mlp as _mlp_lib

F32 = mybir.dt.float32
BF16 = mybir.dt.bfloat16
BS, GS, D, NH = 64, 2001, 128, 4
N = GS // 2                 # 1000
NCORES = 8
BPC = BS // NCORES          # 8 batches per core
PEXT = 2048                 # extended tour positions (3 pads/cycle; grown if needed)
NIDX = 1008                 # padded gather count (>= N, %16 == 0)
IDXW = NIDX // 16           # 63
MLP_CHUNKS = [(0, 512), (512, 488)]

_CACHE = {}


def _chunks():
    out = []
    c0 = 0
    while c0 < PEXT:
        out.append((c0, min(512, PEXT - c0)))
        c0 += 512
    return out


def _build_nc():
    CHUNKS = _chunks()
    nc = bacc.Bacc(None, target_bir_lowering=False, debug=False)
    hemt_d = nc.dram_tensor("hemt", [BPC, 128, PEXT], F32, kind="ExternalInput")
    sig_d = nc.dram_tensor("sig", [BPC, 4, N], F32, kind="ExternalInput")
    pdidx_d = nc.dram_tensor("pdidx", [BPC, 16, 2 * IDXW], mybir.dt.int16, kind="ExternalInput")
    mt_d = nc.dram_tensor("mt", [128, NH * 128], F32, kind="ExternalInput")
    cvec_d = nc.dram_tensor("cvec", [BPC, 128, NH], F32, kind="ExternalInput")
    w1p_d = nc.dram_tensor("w1p", [4, 32], F32, kind="ExternalInput")
    w1d_d = nc.dram_tensor("w1d", [4, 32], F32, kind="ExternalInput")
    w1s_d = nc.dram_tensor("w1s", [4, 32], F32, kind="ExternalInput")
    b1e_d = nc.dram_tensor("b1e", [32, BPC], F32, kind="ExternalInput")
    w2t_d = nc.dram_tensor("w2t", [32, 32], F32, kind="ExternalInput")
    b2_d = nc.dram_tensor("b2", [32, 1], F32, kind="ExternalInput")
    w3t_d = nc.dram_tensor("w3t", [32, 1], F32, kind="ExternalInput")
    b3_d = nc.dram_tensor("b3", [1, 1], F32, kind="ExternalInput")
    hmask_d = nc.dram_tensor("hmask", [128, NH * NH], F32, kind="ExternalInput")
    out_d = nc.dram_tensor("out", [BPC, N], F32, kind="ExternalOutput")

    with tile.TileContext(nc) as tc:
        with ExitStack() as ctx:
            const = ctx.enter_context(tc.tile_pool(name="const", bufs=1))
            inpool = ctx.enter_context(tc.tile_pool(name="inpool", bufs=2))
            epool = ctx.enter_context(
                tc.tile_pool(name="epool", bufs=2 if PEXT <= 2048 else 1))
            ppool = ctx.enter_context(tc.tile_pool(name="ppool", bufs=1))
            spool = ctx.enter_context(tc.tile_pool(name="spool", bufs=2))
            a_ps = ctx.enter_context(tc.tile_pool(name="a_ps", bufs=2, space="PSUM"))
            cs_ps = ctx.enter_context(tc.tile_pool(name="cs_ps", bufs=2, space="PSUM"))
            mlp_ps = ctx.enter_context(tc.tile_pool(name="mlp_ps", bufs=2, space="PSUM"))

            nc.gpsimd.load_library(_mlp_lib)

            hmask_f32 = const.tile([128, NH * NH], F32)
            nc.sync.dma_start(out=hmask_f32[:], in_=hmask_d[:])
            hmask_sb = const.tile([128, NH * NH], BF16)
            nc.vector.tensor_copy(out=hmask_sb[:], in_=hmask_f32[:])
            mt_sb = const.tile([128, NH * 128], F32)
            nc.sync.dma_start(out=mt_sb[:], in_=mt_d[:])
            w1p_sb = const.tile([4, 32], F32)
            nc.sync.dma_start(out=w1p_sb[:], in_=w1p_d[:])
            w1d_sb = const.tile([4, 32], F32)
            nc.sync.dma_start(out=w1d_sb[:], in_=w1d_d[:])
            w1s_sb = const.tile([4, 32], F32)
            nc.sync.dma_start(out=w1s_sb[:], in_=w1s_d[:])
            b1e_sb = const.tile([32, BPC], F32)
            nc.sync.dma_start(out=b1e_sb[:], in_=b1e_d[:])
            w2t_sb = const.tile([32, 32], F32)
            nc.sync.dma_start(out=w2t_sb[:], in_=w2t_d[:])
            b2_sb = const.tile([32, 1], F32)
            nc.sync.dma_start(out=b2_sb[:], in_=b2_d[:])
            w3t_sb = const.tile([32, 1], F32)
            nc.sync.dma_start(out=w3t_sb[:], in_=w3t_d[:])
            b3_sb = const.tile([1, 1], F32)
            nc.sync.dma_start(out=b3_sb[:], in_=b3_d[:])
            compat2 = []
            for i in range(2):
                t = const.tile([16, PEXT], F32, name=f"compat{i}")
                nc.gpsimd.memset(t[:], 0.0)
                compat2.append(t)

            for b in range(BPC):
                pdidx_sb = inpool.tile([16, 2 * IDXW], mybir.dt.int16)
                nc.sync.dma_start(out=pdidx_sb[:], in_=pdidx_d[b])
                cv_sb = inpool.tile([128, NH], F32)
                nc.sync.dma_start(out=cv_sb[:], in_=cvec_d[b])
                sig_sb = inpool.tile([4, N], F32)
                nc.sync.dma_start(out=sig_sb[:], in_=sig_d[b])

                e_fm = epool.tile([128, PEXT], F32)
                nc.sync.dma_start(out=e_fm[:], in_=hemt_d[b])

                # bf16 copies for the 2x-mode product stage
                e_bf = epool.tile([128, PEXT], BF16)
                nc.vector.tensor_copy(out=e_bf[:], in_=e_fm[:])
                # fm1[:, j] = F[:, j+1] = E[:, j+1] - E[:, j+3]  (stored shifted
                # by -1 so every product op is even-aligned for 2x mode)
                fm1 = epool.tile([128, PEXT], BF16)
                nc.vector.tensor_sub(fm1[:, 0:PEXT - 3], e_fm[:, 1:PEXT - 2], e_fm[:, 3:PEXT])
                nc.vector.tensor_copy(out=fm1[:, PEXT - 3:PEXT], in_=e_fm[:, PEXT - 3:PEXT])

                # A'_h = Mt_h^T E + c_h, drained PSUM->SBUF as bf16 (ACT h<3, DVE h=3)
                a_bf = epool.tile([128, NH, PEXT], BF16)
                for h in range(NH):
                    cv = cv_sb[:, h:h + 1]
                    for pair0 in range(0, PEXT, 1024):
                        pw = min(1024, PEXT - pair0)
                        ap = a_ps.tile([128, 1024], F32, space="PSUM", tag="a")
                        for (c0, w) in [(c, min(512, pw - (c - pair0)))
                                        for c in range(pair0, pair0 + pw, 512)]:
                            nc.tensor.matmul(out=ap[:, c0 - pair0:c0 - pair0 + w],
                                             lhsT=mt_sb[:, h * 128:(h + 1) * 128],
                                             rhs=e_fm[:, c0:c0 + w], start=True, stop=True)
                        if h < 3:
                            nc.scalar.add(out=a_bf[:, h, pair0:pair0 + pw],
                                          in_=ap[:, :pw], add=cv)
                        else:
                            nc.vector.tensor_scalar_add(a_bf[:, h, pair0:pair0 + pw],
                                                        ap[:, :pw], cv)

                # products (bf16, 2x): p_sb[:, 2h, j] = A'_h[:, j] * F[:, j+1]
                #                      p_sb[:, 2h+1, j] = A'_h[:, j] * E[:, j+2]
                p_sb = ppool.tile([128, 2 * NH, PEXT], BF16)
                for h in range(NH):
                    nc.vector.tensor_mul(p_sb[:, 2 * h, :], a_bf[:, h, :], fm1[:])
                    nc.vector.tensor_mul(p_sb[:, 2 * h + 1, 0:PEXT - 2],
                                         a_bf[:, h, 0:PEXT - 2], e_bf[:, 2:PEXT])
                    nc.vector.tensor_mul(p_sb[:, 2 * h + 1, PEXT - 2:PEXT],
                                         a_bf[:, h, PEXT - 2:PEXT], e_bf[:, PEXT - 2:PEXT])

                # compat[h, pos p] = colsum(P2_h)[p] + colsum(P1_h)[p-1]
                compat_sb = compat2[b % 2]
                for (c0, w) in CHUNKS:
                    cs = cs_ps.tile([4, 512], F32, space="PSUM", tag="cs")
                    for h in range(NH):
                        mk = hmask_sb[:, h * NH:(h + 1) * NH]
                        if h < NH - 1:
                            nc.tensor.matmul(out=cs[:, :w], lhsT=mk,
                                             rhs=p_sb[:, 2 * h + 1, c0:c0 + w],
                                             start=(h == 0), stop=False)
                            t1 = (cs[:, 1:w], p_sb[:, 2 * h, 0:w - 1]) if c0 == 0 else \
                                 (cs[:, :w], p_sb[:, 2 * h, c0 - 1:c0 - 1 + w])
                            nc.tensor.matmul(out=t1[0], lhsT=mk, rhs=t1[1],
                                             start=False, stop=False,
                                             skip_group_check=True)
                        else:
                            t1 = (cs[:, 1:w], p_sb[:, 2 * h, 0:w - 1]) if c0 == 0 else \
                                 (cs[:, :w], p_sb[:, 2 * h, c0 - 1:c0 - 1 + w])
                            nc.tensor.matmul(out=t1[0], lhsT=mk, rhs=t1[1],
                                             start=False, stop=False,
                                             skip_group_check=True)
                            nc.tensor.matmul(out=cs[:, :w], lhsT=mk,
                                             rhs=p_sb[:, 2 * h + 1, c0:c0 + w],
                                             start=False, stop=True)
                    nc.scalar.copy(out=compat_sb[0:4, c0:c0 + w], in_=cs[:, :w])

                pd_g = spool.tile([16, 2 * NIDX], F32)
                nc.gpsimd.ap_gather(pd_g[:], compat_sb[:], pdidx_sb[:],
                                    channels=16, num_elems=PEXT, d=1, num_idxs=2 * NIDX)

                x1_sb = spool.tile([32, N], F32)
                x2_sb = spool.tile([32, N], F32)
                tab_sb = spool.tile([1, N], F32)
                for (c0, w) in MLP_CHUNKS:
                    x1p = mlp_ps.tile([32, 512], F32, space="PSUM", tag="m")
                    nc.tensor.matmul(out=x1p[:, :w], lhsT=w1p_sb[:], rhs=pd_g[0:4, c0:c0 + w],
                                     start=True, stop=False)
                    nc.tensor.matmul(out=x1p[:, :w], lhsT=w1d_sb[:], rhs=pd_g[0:4, NIDX + c0:NIDX + c0 + w],
                                     start=False, stop=False)
                    nc.tensor.matmul(out=x1p[:, :w], lhsT=w1s_sb[:], rhs=sig_sb[:, c0:c0 + w],
                                     start=False, stop=True)
                    nc.scalar.activation(out=x1_sb[:, c0:c0 + w], in_=x1p[:, :w],
                                         func=mybir.ActivationFunctionType.Relu,
                                         bias=b1e_sb[:, b:b + 1], scale=1.0)
                for (c0, w) in MLP_CHUNKS:
                    x2p = mlp_ps.tile([32, 512], F32, space="PSUM", tag="m")
                    nc.tensor.matmul(out=x2p[:, :w], lhsT=w2t_sb[:], rhs=x1_sb[:, c0:c0 + w],
                                     start=True, stop=True)
                    nc.scalar.activation(out=x2_sb[:, c0:c0 + w], in_=x2p[:, :w],
                                         func=mybir.ActivationFunctionType.Relu,
                                         bias=b2_sb[:], scale=1.0)
                for (c0, w) in MLP_CHUNKS:
                    tp3 = mlp_ps.tile([1, 512], F32, space="PSUM", tag="m")
                    nc.tensor.matmul(out=tp3[:, :w], lhsT=w3t_sb[:], rhs=x2_sb[:, c0:c0 + w],
                                     start=True, stop=True)
                    nc.scalar.activation(out=tab_sb[:, c0:c0 + w], in_=tp3[:, :w],
                                         func=mybir.ActivationFunctionType.Tanh,
                                         bias=b3_sb[:], scale=1.0)

                # softmax over 6*tanh; values bounded in [-6, 6] so no max-shift needed
                ex_sb = spool.tile([1, N], F32)
                ssum = spool.tile([1, 1], F32)
                nc.scalar.activation(out=ex_sb[:], in_=tab_sb[:],
                                     func=mybir.ActivationFunctionType.Exp,
                                     bias=0.0, scale=6.0, accum_out=ssum[:])
                rcp = spool.tile([1, 1], F32)
                nc.vector.reciprocal(rcp[:], ssum[:])
                probs = spool.tile([1, N], F32)
                nc.vector.tensor_scalar_mul(probs[:], ex_sb[:], rcp[:])
                nc.sync.dma_start(out=out_d[b:b + 1, :], in_=probs[:])
    nc.compile()
    return nc


def _decompose(perm):
    visited = np.zeros(GS, bool)
    order = []
    real = []
    for start in range(GS):
        if visited[start]:
            continue
        cyc = [start]
        visited[start] = True
        nxt = int(perm[start])
        while nxt != start:
            cyc.append(nxt)
            visited[nxt] = True
            nxt = int(perm[nxt])
        L = len(cyc)
        order.extend([cyc[-1]] + cyc + [cyc[0 % L], cyc[1 % L]])
        real.extend([False] + [True] * L + [False] * 2)
    assert len(order) <= PEXT, f"too many cycles: ext len {len(order)}"
    pad = PEXT - len(order)
    order.extend([0] * pad)
    real.extend([False] * pad)
    return np.asarray(order, np.int64), np.asarray(real, bool)


def _ext_len(perm):
    visited = np.zeros(GS, bool)
    ncyc = 0
    for start in range(GS):
        if not visited[start]:
            ncyc += 1
            visited[start] = True
            nxt = int(perm[start])
            while nxt != start:
                visited[nxt] = True
                nxt = int(perm[nxt])
    return GS + 3 * ncyc


def _idx_tile(ppos, dpos):
    idx = np.zeros(2 * NIDX, np.int16)
    idx[:ppos.shape[0]] = ppos
    idx[NIDX:NIDX + dpos.shape[0]] = dpos
    return idx.reshape(2 * IDXW, 16).T.copy()


def _host_prep(inputs):
    h_em = np.asarray(inputs["h_em"], np.float32)
    rec = np.asarray(inputs["rec"], np.int64)
    sig = np.ascontiguousarray(np.asarray(inputs["selection_sig"], np.float32))
    Wn = np.asarray(inputs["W_node"], np.float64)
    Wg = np.asarray(inputs["W_graph"], np.float64)
    WQ = np.asarray(inputs["W_Q"], np.float64)
    WK = np.asarray(inputs["W_K"], np.float64)
    w1 = np.asarray(inputs["agg_w1"], np.float64)
    b1 = np.asarray(inputs["agg_b1"], np.float64)
    w2 = np.asarray(inputs["agg_w2"], np.float32)
    b2 = np.asarray(inputs["agg_b2"], np.float32)
    w3 = np.asarray(inputs["agg_w3"], np.float32)
    b3 = np.asarray(inputs["agg_b3"], np.float32)

    Mt = np.zeros((NH, D, D), np.float64)
    C = np.zeros((NH, D, D), np.float64)
    S = np.zeros((NH, D, D), np.float64)
    for h in range(NH):
        M = WQ[h] @ WK[h].T
        Mt[h] = Wn.T @ M @ Wn
        C[h] = Wn.T @ (M + M.T) @ Wg
        S[h] = Wg.T @ M @ Wg
    mt = np.concatenate([Mt[h].astype(np.float32) for h in range(NH)], axis=1)

    g = h_em.max(axis=1).astype(np.float64)                      # (BS, D)
    cvec = np.einsum("hdf,bf->bdh", C, g).astype(np.float32)     # (BS, D, NH)
    svec = np.einsum("bd,hdf,bf->bh", g, S, g)                   # (BS, NH)
    b1_eff = (b1[None, :] + svec @ (w1[:, 0:4] + w1[:, 4:8]).T).astype(np.float32)

    w1f = w1.astype(np.float32)
    hmask = np.zeros((128, NH * NH), np.float32)
    for h in range(NH):
        hmask[:, h * NH + h] = 1.0
    shared = {
        "mt": mt,
        "hmask": hmask,
        "w1p": np.ascontiguousarray(w1f[:, 0:4].T),
        "w1d": np.ascontiguousarray(w1f[:, 4:8].T),
        "w1s": np.ascontiguousarray(w1f[:, 8:12].T),
        "w2t": np.ascontiguousarray(w2.T),
        "b2": b2.reshape(32, 1),
        "w3t": np.ascontiguousarray(w3.T),
        "b3": b3.reshape(1, 1),
    }

    in_maps = []
    for core in range(NCORES):
        b0 = core * BPC
        hemt = np.empty((BPC, 128, PEXT), np.float32)
        pdidx = np.empty((BPC, 16, 2 * IDXW), np.int16)
        for bl in range(BPC):
            order, real = _decompose(rec[b0 + bl])
            hemt[bl] = h_em[b0 + bl][order].T
            pon = np.zeros(GS, np.int64)
            pon[order[real]] = np.nonzero(real)[0]
            pdidx[bl] = _idx_tile(pon[1:N + 1], pon[N + 1:2 * N + 1])
        m = {
            "hemt": hemt,
            "sig": sig[b0:b0 + BPC],
            "pdidx": pdidx,
            "cvec": cvec[b0:b0 + BPC],
            "b1e": np.ascontiguousarray(b1_eff[b0:b0 + BPC].T),
        }
        m.update(shared)
        in_maps.append(m)
    return in_maps


def kernel(**inputs) -> np.ndarray:
    global PEXT
    rec = np.asarray(inputs["rec"], np.int64)
    need = max(_ext_len(rec[b]) for b in range(rec.shape[0]))
    want = max(2048, -(-need // 512) * 512)
    if want != PEXT or "nc" not in _CACHE:
        PEXT = want
        _CACHE["nc"] = _build_nc()
    nc = _CACHE["nc"]
    in_maps = _host_prep(inputs)
    res = run_bass_kernel_spmd(nc, in_maps, list(range(NCORES)))
    return np.concatenate([res.results[i]["out"] for i in range(NCORES)], axis=0)



# revision 48
# speedup vs baseline: 2.0928x; 2.0928x over previous
"""Trainium2 Bass kernel for nn_MultiHeadDecoder (sparse neighbour compat + MLP + softmax).

Strategy (data-parallel over batch, 8 batches per core):
 - Host: decompose the `rec` permutation into cycles and lay nodes out in tour
   order (with per-cycle pad columns) so predecessor / succ^2 lookups become
   free-dim column shifts on-chip.  The per-core shard is shipped pre-gathered
   and feature-major in fp16: hemt[b] = h_em[b][order].T ([128, PEXT+2], two
   pad columns so the +2-shift product needs no tail fixup), along with
   fm1[b][:, j] = E[:, j+1] - E[:, j+3] (the shifted first-product factor,
   computed host-side in f32 then cast to fp16).
 - Algebra folding (host, float64): per-head Q/K projections collapse to one
   bilinear form per head:
       compat[pos p] = (A_h[:,p-1]+c_h).F[:,p] + (A_h[:,p]+c_h).E[:,p+2]  (+s)
   where A_h = Mt_h^T E, F = E - shift2(E), per-batch scalars c_h ride the
   PSUM->SBUF drain and s folds into the first MLP bias.
 - Device per batch: 4 fp16 128x128xPEXT matmuls (A heads), PSUM->SBUF drains
   with +c (ACT/DVE split), 8 fp16 2x-mode products (7 DVE + 1 GPSIMD),
   TensorE column-sum (mask lhsT) into compat, GPSIMD ap_gather to node
   order, 12->32->32->1 MLP (fp16 throughout; the x2 relu drains two batches' stacked PSUM at once).  The
   scalar tail (tanh, exp+accum, reciprocal, renorm) is batched: a
   block-diagonal L3 yields 4 batches' scores per matmul and the tail runs
   once on [8, N] instead of 8x on [1, N].
 - Emission is software-pipelined (front(b) | colsum/gather(b-1) | mlp(b-2))
   with per-tensor double-buffered tile rings so the in-order engines always
   have independent work queued.
"""
import os
import sys
from contextlib import ExitStack

import numpy as np

for _p in ("/opt/trn_rl_repo", "/root/.axon_site/_ro/trn_rl_repo"):
    if os.path.isdir(_p) and _p not in sys.path:
        sys.path.insert(0, _p)

import concourse.bacc as bacc
import concourse.bass as bass
import concourse.mybir as mybir
import concourse.tile as tile
from concourse.bass_utils import run_bass_kernel_spmd
from concourse.library_config import mlp as _mlp_lib

F32 = mybir.dt.float32
F32R = mybir.dt.float32r
F16 = mybir.dt.float16
BS, GS, D, NH = 64, 2001, 128, 4
N = GS // 2                 # 1000
NCORES = 8
BPC = BS // NCORES          # 8 batches per core
PEXT = 2048                 # extended tour positions (3 pads/cycle; grown if needed)
NIDX = 1008                 # padded gather count (>= N, %16 == 0)
IDXW = NIDX // 16           # 63
MLP_CHUNKS = [(0, 512), (512, 488)]
RELU = mybir.ActivationFunctionType.Relu
TANH = mybir.ActivationFunctionType.Tanh
EXP = mybir.ActivationFunctionType.Exp

_CACHE = {}


def _chunks():
    out = []
    c0 = 0
    while c0 < PEXT:
        out.append((c0, min(512, PEXT - c0)))
        c0 += 512
    return out


def _build_nc():
    CHUNKS = _chunks()
    nc = bacc.Bacc(None, target_bir_lowering=False, debug=False)
    hemt_d = nc.dram_tensor("hemt", [BPC, 128, PEXT + 2], F16, kind="ExternalInput")
    fm1_d = nc.dram_tensor("fm1", [BPC, 128, PEXT], F16, kind="ExternalInput")
    sig_d = nc.dram_tensor("sig", [BPC, 4, N], F32, kind="ExternalInput")
    pdidx_d = nc.dram_tensor("pdidx", [BPC, 16, 2 * IDXW], mybir.dt.int16, kind="ExternalInput")
    mt_d = nc.dram_tensor("mt", [128, NH * 128], F16, kind="ExternalInput")
    cvec_d = nc.dram_tensor("cvec", [BPC, 128, NH], F32, kind="ExternalInput")
    w1e_d = nc.dram_tensor("w1e", [12, 32], F16, kind="ExternalInput")
    b1e_d = nc.dram_tensor("b1e", [32, BPC], F32, kind="ExternalInput")
    w2t_d = nc.dram_tensor("w2t", [32, 32], F16, kind="ExternalInput")
    b2_d = nc.dram_tensor("b2", [64, 1], F32, kind="ExternalInput")
    w3blk_d = nc.dram_tensor("w3blk", [128, 4], F16, kind="ExternalInput")
    b3_d = nc.dram_tensor("b3", [8, 1], F32, kind="ExternalInput")
    hmask_d = nc.dram_tensor("hmask", [128, NH * NH], F16, kind="ExternalInput")
    out_d = nc.dram_tensor("out", [BPC, N], F32, kind="ExternalOutput")

    with tile.TileContext(nc) as tc:
        with ExitStack() as ctx:
            const = ctx.enter_context(tc.tile_pool(name="const", bufs=1))
            inpool = ctx.enter_context(tc.tile_pool(name="inpool", bufs=2))
            work = ctx.enter_context(tc.tile_pool(name="work", bufs=2))
            acc = ctx.enter_context(tc.tile_pool(name="acc", bufs=1))
            a_ps = ctx.enter_context(tc.tile_pool(name="a_ps", bufs=2, space="PSUM"))
            cs_ps = ctx.enter_context(tc.tile_pool(name="cs_ps", bufs=2, space="PSUM"))
            mlp_ps = ctx.enter_context(tc.tile_pool(name="mlp_ps", bufs=2, space="PSUM"))

            nc.gpsimd.load_library(_mlp_lib)

            # mt first on the queue (A-proj(0) needs it); the small consts
            # are emitted after front(0) so batch 0's big loads aren't queued
            # behind them
            mt_sb = const.tile([128, NH * 128], F16)
            nc.sync.dma_start(out=mt_sb[:], in_=mt_d[:])
            hmask_sb = const.tile([128, NH * NH], F16)
            w1e_sb = const.tile([12, 32], F16)
            b1e_sb = const.tile([32, BPC], F32)
            w2t_sb = const.tile([32, 32], F16)
            b2_sb = const.tile([64, 1], F32)
            w3blk_sb = const.tile([128, 4], F16)
            b3_sb = const.tile([8, 1], F32)

            def load_small_consts():
                nc.sync.dma_start(out=hmask_sb[:], in_=hmask_d[:])
                nc.sync.dma_start(out=w1e_sb[:], in_=w1e_d[:])
                nc.sync.dma_start(out=b1e_sb[:], in_=b1e_d[:])
                nc.sync.dma_start(out=w2t_sb[:], in_=w2t_d[:])
                nc.sync.dma_start(out=b2_sb[:], in_=b2_d[:])
                nc.sync.dma_start(out=w3blk_sb[:], in_=w3blk_d[:])
                nc.sync.dma_start(out=b3_sb[:], in_=b3_d[:])
            compat2 = []
            for i in range(2):
                t = const.tile([16, PEXT], F32, name=f"compat{i}")
                nc.gpsimd.memset(t[:], 0.0)
                compat2.append(t)

            # batched scalar-tail accumulators: tanh drains each 4-batch
            # group's L3 scores here; exp/renorm run once per [4, N] group
            tab_g = [acc.tile([4, N], F32, name="tab0", tag="tab0"),
                     acc.tile([4, N], F32, name="tab1", tag="tab1")]

            st = [dict() for _ in range(BPC)]
            mlp_state = {}

            def dma_in(b):
                s = st[b]
                pdidx_sb = inpool.tile([16, 2 * IDXW], mybir.dt.int16,
                                       tag="idx", bufs=4, name=f"pdidx{b}")
                nc.sync.dma_start(out=pdidx_sb[:], in_=pdidx_d[b])
                cv_sb = inpool.tile([128, NH], F32, tag="cv", bufs=3, name=f"cv{b}")
                nc.sync.dma_start(out=cv_sb[:], in_=cvec_d[b])
                e_bf = work.tile([128, PEXT + 2], F16, tag="e", bufs=3,
                                 name=f"e{b}")
                nc.sync.dma_start(out=e_bf[:], in_=hemt_d[b])
                fm1 = work.tile([128, PEXT], F16, tag="f", bufs=3, name=f"f{b}")
                nc.sync.dma_start(out=fm1[:], in_=fm1_d[b])
                s["e"] = e_bf
                s["f"] = fm1
                s["cv"] = cv_sb
                s["pdidx"] = pdidx_sb

            def front(b):
                s = st[b]
                e_bf, fm1, cv_sb = s["e"], s["f"], s["cv"]

                # A'_h = Mt_h^T E + c_h (bf16 matmul), drained PSUM->SBUF as
                # bf16 with the +c folded in (ACT / DVE split for balance)
                a_bf = work.tile([128, NH, PEXT], F16, tag="abf", name=f"a{b}")
                for h in range(NH):
                    cv = cv_sb[:, h:h + 1]
                    for pair0 in range(0, PEXT, 1024):
                        pw = min(1024, PEXT - pair0)
                        ap = a_ps.tile([128, 1024], F32, space="PSUM", tag="a",
                                       name=f"ap{b}_{h}_{pair0}")
                        for (c0, w) in [(c, min(512, pw - (c - pair0)))
                                        for c in range(pair0, pair0 + pw, 512)]:
                            nc.tensor.matmul(out=ap[:, c0 - pair0:c0 - pair0 + w],
                                             lhsT=mt_sb[:, h * 128:(h + 1) * 128],
                                             rhs=e_bf[:, c0:c0 + w], start=True, stop=True)
                        if h < 3:
                            nc.scalar.add(out=a_bf[:, h, pair0:pair0 + pw],
                                          in_=ap[:, :pw], add=cv)
                        else:
                            nc.vector.tensor_scalar_add(a_bf[:, h, pair0:pair0 + pw],
                                                        ap[:, :pw], cv)

                # products (bf16, 2x): p_sb[:, 2h, j] = A'_h[:, j] * F[:, j+1]
                #                      p_sb[:, 2h+1, j] = A'_h[:, j] * E[:, j+2]
                # (one of the 8 rides the otherwise-idle GPSIMD)
                p_sb = work.tile([128, 2 * NH, PEXT], F16, tag="p", name=f"p{b}")
                nc.gpsimd.tensor_mul(p_sb[:, 0, :], a_bf[:, 0, :], fm1[:])
                for h in range(NH):
                    if h > 0:
                        nc.vector.tensor_mul(p_sb[:, 2 * h, :], a_bf[:, h, :], fm1[:])
                    nc.vector.tensor_mul(p_sb[:, 2 * h + 1, :],
                                         a_bf[:, h, :], e_bf[:, 2:PEXT + 2])
                s["p"] = p_sb

            def mid(b):
                s = st[b]
                p_sb = s["p"]
                # compat[h, pos p] = colsum(P2_h)[p] + colsum(P1_h)[p-1]
                compat_sb = compat2[b % 2]
                for (c0, w) in CHUNKS:
                    cs = cs_ps.tile([4, 512], F32, space="PSUM", tag="cs",
                                    name=f"cs{b}_{c0}")
                    for h in range(NH):
                        mk = hmask_sb[:, h * NH:(h + 1) * NH]
                        if h < NH - 1:
                            nc.tensor.matmul(out=cs[:, :w], lhsT=mk,
                                             rhs=p_sb[:, 2 * h + 1, c0:c0 + w],
                                             start=(h == 0), stop=False)
                            t1 = (cs[:, 1:w], p_sb[:, 2 * h, 0:w - 1]) if c0 == 0 else \
                                 (cs[:, :w], p_sb[:, 2 * h, c0 - 1:c0 - 1 + w])
                            nc.tensor.matmul(out=t1[0], lhsT=mk, rhs=t1[1],
                                             start=False, stop=False,
                                             skip_group_check=True)
                        else:
                            t1 = (cs[:, 1:w], p_sb[:, 2 * h, 0:w - 1]) if c0 == 0 else \
                                 (cs[:, :w], p_sb[:, 2 * h, c0 - 1:c0 - 1 + w])
                            nc.tensor.matmul(out=t1[0], lhsT=mk, rhs=t1[1],
                                             start=False, stop=False,
                                             skip_group_check=True)
                            nc.tensor.matmul(out=cs[:, :w], lhsT=mk,
                                             rhs=p_sb[:, 2 * h + 1, c0:c0 + w],
                                             start=False, stop=True)
                    nc.scalar.copy(out=compat_sb[0:4, c0:c0 + w], in_=cs[:, :w])

                pd_g = work.tile([16, 2 * NIDX], F32, tag="pdg", name=f"pdg{b}")
                nc.gpsimd.ap_gather(pd_g[:], compat_sb[:], s["pdidx"][:],
                                    channels=16, num_elems=PEXT, d=1, num_idxs=2 * NIDX)
                # pack [pickup(4); delivery(4); sig(4)] as a single 12-row
                # MLP input so L1 is one fp32r matmul per chunk
                pd12f = work.tile([12, N], F32, tag="pd12f", name=f"pd12f_{b}")
                nc.sync.dma_start(out=pd12f[0:4, :], in_=pd_g[0:4, 0:N])
                nc.sync.dma_start(out=pd12f[4:8, :], in_=pd_g[0:4, NIDX:NIDX + N])
                nc.sync.dma_start(out=pd12f[8:12, :], in_=sig_d[b])
                pd12 = work.tile([12, N], F16, tag="pd12", name=f"pd12_{b}")
                nc.gpsimd.tensor_copy(out=pd12[:], in_=pd12f[:])
                s["pd12"] = pd12

            def mlp(b):
                s = st[b]
                pd12 = s["pd12"]
                if b % 4 == 0:
                    mlp_state["x2g"] = work.tile([128, N], F16, tag="x2g",
                                                 name=f"x2g{b // 4}")
                x2g = mlp_state["x2g"]
                p = b % 2
                if p == 0:
                    mlp_state["x2p"] = []
                x2p_t = mlp_state["x2p"]
                x1_sb = work.tile([32, N], F16, tag="x1", name=f"x1_{b}")
                for (c0, w) in MLP_CHUNKS:
                    x1p = cs_ps.tile([32, 512], F32, space="PSUM", tag="cs",
                                     name=f"x1p{b}_{c0}")
                    nc.tensor.matmul(out=x1p[:, :w], lhsT=w1e_sb[:],
                                     rhs=pd12[:, c0:c0 + w], start=True, stop=True)
                    nc.scalar.activation(out=x1_sb[:, c0:c0 + w], in_=x1p[:, :w],
                                         func=RELU, bias=b1e_sb[:, b:b + 1], scale=1.0)
                # L2 writes a [64, .] PSUM pair tile (even batch rows 0-31,
                # odd rows 32-63); one relu drains both batches at once
                for ci, (c0, w) in enumerate(MLP_CHUNKS):
                    if p == 0:
                        x2p_t.append(mlp_ps.tile([64, 512], F32, space="PSUM",
                                                 tag="m", name=f"x2p{b}_{ci}"))
                    nc.tensor.matmul(out=x2p_t[ci][32 * p:32 * p + 32, :w],
                                     lhsT=w2t_sb[:], rhs=x1_sb[:, c0:c0 + w],
                                     start=True, stop=True)
                if p == 1:
                    r0 = 64 * ((b % 4) // 2)
                    for ci, (c0, w) in enumerate(MLP_CHUNKS):
                        nc.scalar.activation(out=x2g[r0:r0 + 64, c0:c0 + w],
                                             in_=x2p_t[ci][:, :w],
                                             func=RELU, bias=b2_sb[:], scale=1.0)
                if b % 4 == 3:
                    # block-diagonal L3: one matmul yields 4 batches' scores
                    g0 = 4 * (b // 4)
                    for (c0, w) in MLP_CHUNKS:
                        l3t = cs_ps.tile([4, 512], F32, space="PSUM", tag="cs",
                                         name=f"l3t{b}_{c0}")
                        nc.tensor.matmul(out=l3t[:, :w], lhsT=w3blk_sb[:],
                                         rhs=x2g[:, c0:c0 + w], start=True, stop=True)
                        nc.scalar.activation(out=tab_g[b // 4][:, c0:c0 + w],
                                             in_=l3t[:, :w], func=TANH,
                                             bias=b3_sb[0:4, :], scale=1.0)

            # software-pipelined emission: keeps independent work in front of
            # every in-order engine queue
            dma_in(0)
            dma_in(1)
            load_small_consts()
            for i in range(BPC + 2):
                if i < BPC:
                    front(i)
                if i + 2 < BPC:
                    dma_in(i + 2)
                if 1 <= i <= BPC:
                    mid(i - 1)
                if i >= 2:
                    mlp(i - 2)

            # batched scalar tail, one pass per 4-batch group:
            # probs = softmax(6 * tanh(score + b3), axis=-1)
            for g in range(2):
                ex_sb = work.tile([4, N], F32, tag=f"ex{g}", bufs=1,
                                  name=f"ex{g}")
                ssum = work.tile([4, 1], F32, tag=f"ssum{g}", bufs=1,
                                 name=f"ssum{g}")
                nc.scalar.activation(out=ex_sb[:], in_=tab_g[g][:],
                                     func=EXP, bias=0.0, scale=6.0,
                                     accum_out=ssum[:])
                rcp = work.tile([4, 1], F32, tag=f"rcp{g}", bufs=1,
                                name=f"rcp{g}")
                nc.vector.reciprocal(rcp[:], ssum[:])
                probs = work.tile([4, N], F32, tag=f"probs{g}", bufs=1,
                                  name=f"probs{g}")
                nc.vector.tensor_scalar_mul(probs[:], ex_sb[:], rcp[:])
                nc.sync.dma_start(out=out_d[4 * g:4 * g + 4, :], in_=probs[:])
    nc.compile()
    return nc


def _decompose(perm):
    visited = np.zeros(GS, bool)
    order = []
    real = []
    for start in range(GS):
        if visited[start]:
            continue
        cyc = [start]
        visited[start] = True
        nxt = int(perm[start])
        while nxt != start:
            cyc.append(nxt)
            visited[nxt] = True
            nxt = int(perm[nxt])
        L = len(cyc)
        order.extend([cyc[-1]] + cyc + [cyc[0 % L], cyc[1 % L]])
        real.extend([False] + [True] * L + [False] * 2)
    assert len(order) <= PEXT, f"too many cycles: ext len {len(order)}"
    pad = PEXT - len(order)
    order.extend([0] * pad)
    real.extend([False] * pad)
    return np.asarray(order, np.int64), np.asarray(real, bool)


def _ext_len(perm):
    visited = np.zeros(GS, bool)
    ncyc = 0
    for start in range(GS):
        if not visited[start]:
            ncyc += 1
            visited[start] = True
            nxt = int(perm[start])
            while nxt != start:
                visited[nxt] = True
                nxt = int(perm[nxt])
    return GS + 3 * ncyc


def _idx_tile(ppos, dpos):
    idx = np.zeros(2 * NIDX, np.int16)
    idx[:ppos.shape[0]] = ppos
    idx[NIDX:NIDX + dpos.shape[0]] = dpos
    return idx.reshape(2 * IDXW, 16).T.copy()


def _f16(x):
    return np.asarray(x, np.float32).astype(np.float16)


def _host_prep(inputs):
    h_em = np.asarray(inputs["h_em"], np.float32)
    rec = np.asarray(inputs["rec"], np.int64)
    sig = np.ascontiguousarray(np.asarray(inputs["selection_sig"], np.float32))
    Wn = np.asarray(inputs["W_node"], np.float64)
    Wg = np.asarray(inputs["W_graph"], np.float64)
    WQ = np.asarray(inputs["W_Q"], np.float64)
    WK = np.asarray(inputs["W_K"], np.float64)
    w1 = np.asarray(inputs["agg_w1"], np.float64)
    b1 = np.asarray(inputs["agg_b1"], np.float64)
    w2 = np.asarray(inputs["agg_w2"], np.float32)
    b2 = np.asarray(inputs["agg_b2"], np.float32)
    w3 = np.asarray(inputs["agg_w3"], np.float32)
    b3 = np.asarray(inputs["agg_b3"], np.float32)

    Mt = np.zeros((NH, D, D), np.float64)
    C = np.zeros((NH, D, D), np.float64)
    S = np.zeros((NH, D, D), np.float64)
    for h in range(NH):
        M = WQ[h] @ WK[h].T
        Mt[h] = Wn.T @ M @ Wn
        C[h] = Wn.T @ (M + M.T) @ Wg
        S[h] = Wg.T @ M @ Wg
    mt = np.concatenate([Mt[h] for h in range(NH)], axis=1)

    g = h_em.max(axis=1).astype(np.float64)                      # (BS, D)
    cvec = np.einsum("hdf,bf->bdh", C, g).astype(np.float32)     # (BS, D, NH)
    svec = np.einsum("bd,hdf,bf->bh", g, S, g)                   # (BS, NH)
    b1_eff = (b1[None, :] + svec @ (w1[:, 0:4] + w1[:, 4:8]).T).astype(np.float32)

    w1f = w1.astype(np.float32)
    hmask = np.zeros((128, NH * NH), np.float32)
    for h in range(NH):
        hmask[:, h * NH + h] = 1.0
    w1e = np.concatenate([w1f[:, 0:4].T, w1f[:, 4:8].T, w1f[:, 8:12].T], axis=0)
    shared = {
        "mt": _f16(mt),
        "hmask": _f16(hmask),
        "w1e": _f16(w1e),
        "w2t": _f16(np.ascontiguousarray(w2.T)),
        "b2": np.tile(b2.reshape(32, 1), (2, 1)),
        "w3blk": _f16(np.kron(np.eye(4), w3.reshape(32, 1))),
        "b3": np.broadcast_to(b3.reshape(1, 1), (BPC, 1)).copy(),
    }

    in_maps = []
    for core in range(NCORES):
        b0 = core * BPC
        hemt = np.zeros((BPC, 128, PEXT + 2), np.float32)
        pdidx = np.empty((BPC, 16, 2 * IDXW), np.int16)
        for bl in range(BPC):
            order, real = _decompose(rec[b0 + bl])
            hemt[bl, :, :PEXT] = h_em[b0 + bl][order].T
            pon = np.zeros(GS, np.int64)
            pon[order[real]] = np.nonzero(real)[0]
            pdidx[bl] = _idx_tile(pon[1:N + 1], pon[N + 1:2 * N + 1])
        fm1 = np.empty((BPC, 128, PEXT), np.float32)
        fm1[:, :, :PEXT - 3] = hemt[:, :, 1:PEXT - 2] - hemt[:, :, 3:PEXT]
        fm1[:, :, PEXT - 3:] = hemt[:, :, PEXT - 3:PEXT]
        m = {
            "hemt": _f16(hemt),
            "fm1": _f16(fm1),
            "sig": sig[b0:b0 + BPC],
            "pdidx": pdidx,
            "cvec": cvec[b0:b0 + BPC],
            "b1e": np.ascontiguousarray(b1_eff[b0:b0 + BPC].T),
        }
        m.update(shared)
        in_maps.append(m)
    return in_maps


def kernel(**inputs) -> np.ndarray:
    global PEXT
    rec = np.asarray(inputs["rec"], np.int64)
    need = max(_ext_len(rec[b]) for b in range(rec.shape[0]))
    want = max(2048, -(-need // 512) * 512)
    if want != PEXT or "nc" not in _CACHE:
        PEXT = want
        _CACHE["nc"] = _build_nc()
    nc = _CACHE["nc"]
    in_maps = _host_prep(inputs)
    res = run_bass_kernel_spmd(nc, in_maps, list(range(NCORES)))
    return np.concatenate([res.results[i]["out"] for i in range(NCORES)], axis=0)


# revision 70
# speedup vs baseline: 2.1172x; 1.0117x over previous
"""Trainium2 Bass kernel for nn_MultiHeadDecoder (sparse neighbour compat + MLP + softmax).

Strategy (data-parallel over batch, 8 batches per core):
 - Host: decompose the `rec` permutation into cycles and lay nodes out in tour
   order (with per-cycle pad columns) so predecessor / succ^2 lookups become
   free-dim column shifts on-chip.  The per-core shard is shipped pre-gathered
   and feature-major in fp16: hemt[b] = h_em[b][order].T ([128, PEXT+2], two
   pad columns so the +2-shift product needs no tail fixup), along with
   fm1[b][:, j] = E[:, j+1] - E[:, j+3] (the shifted first-product factor,
   computed host-side in f32 then cast to fp16).
 - Algebra folding (host, float64): per-head Q/K projections collapse to one
   bilinear form per head:
       compat[pos p] = (A_h[:,p-1]+c_h).F[:,p] + (A_h[:,p]+c_h).E[:,p+2]  (+s)
   where A_h = Mt_h^T E, F = E - shift2(E), per-batch scalars c_h ride the
   PSUM->SBUF drain and s folds into the first MLP bias.
 - Device per batch: 4 fp16 128x128xPEXT matmuls (A heads), PSUM->SBUF drains
   with +c (ACT/DVE split), 8 fp16 2x-mode products (7 DVE + 1 GPSIMD),
   TensorE column-sum (mask lhsT) into compat, GPSIMD ap_gather to node
   order, 12->32->32->1 MLP (fp16 throughout; the x2 relu drains two batches' stacked PSUM at once).  The
   scalar tail (tanh, exp+accum, reciprocal, renorm) is batched: a
   block-diagonal L3 yields 4 batches' scores per matmul and the tail runs
   once on [8, N] instead of 8x on [1, N].
 - Emission is software-pipelined (front(b) | colsum/gather(b-1) | mlp(b-2))
   with per-tensor double-buffered tile rings so the in-order engines always
   have independent work queued.
"""
import os
import sys
from contextlib import ExitStack

import numpy as np

for _p in ("/opt/trn_rl_repo", "/root/.axon_site/_ro/trn_rl_repo"):
    if os.path.isdir(_p) and _p not in sys.path:
        sys.path.insert(0, _p)

import concourse.bacc as bacc
import concourse.bass as bass
import concourse.mybir as mybir
import concourse.tile as tile
from concourse.bass_utils import run_bass_kernel_spmd
from concourse.library_config import mlp as _mlp_lib

F32 = mybir.dt.float32
F32R = mybir.dt.float32r
F16 = mybir.dt.float16
BS, GS, D, NH = 64, 2001, 128, 4
N = GS // 2                 # 1000
NCORES = 8
BPC = BS // NCORES          # 8 batches per core
PEXT = 2048                 # extended tour positions (3 pads/cycle; grown if needed)
NIDX = 1008                 # padded gather count (>= N, %16 == 0)
IDXW = NIDX // 16           # 63
MLP_CHUNKS = [(0, 512), (512, 488)]
RELU = mybir.ActivationFunctionType.Relu
TANH = mybir.ActivationFunctionType.Tanh
EXP = mybir.ActivationFunctionType.Exp

_CACHE = {}


def _chunks():
    out = []
    c0 = 0
    while c0 < PEXT:
        out.append((c0, min(512, PEXT - c0)))
        c0 += 512
    return out


def _build_nc():
    CHUNKS = _chunks()
    big = PEXT > 2048
    EF_BUFS = 2 if big else 3
    AP_BUFS = 1 if big else 2
    PD_BUFS = 2 if big else 3
    nc = bacc.Bacc(None, target_bir_lowering=False, debug=False)
    hemt_d = nc.dram_tensor("hemt", [BPC, 128, PEXT + 2], F16, kind="ExternalInput")
    fm1_d = nc.dram_tensor("fm1", [BPC, 128, PEXT], F16, kind="ExternalInput")
    sig_d = nc.dram_tensor("sig", [BPC, 4, N], F32, kind="ExternalInput")
    pdidx_d = nc.dram_tensor("pdidx", [BPC, 16, 2 * IDXW], mybir.dt.int16, kind="ExternalInput")
    mt_d = nc.dram_tensor("mt", [128, NH * 128], F16, kind="ExternalInput")
    cvec_d = nc.dram_tensor("cvec", [BPC, 128, NH], F32, kind="ExternalInput")
    w1e_d = nc.dram_tensor("w1e", [12, 32], F16, kind="ExternalInput")
    b1e_d = nc.dram_tensor("b1e", [32, BPC], F32, kind="ExternalInput")
    w2t_d = nc.dram_tensor("w2t", [32, 32], F16, kind="ExternalInput")
    b2_d = nc.dram_tensor("b2", [64, 1], F32, kind="ExternalInput")
    w3blk_d = nc.dram_tensor("w3blk", [128, 4], F16, kind="ExternalInput")
    b3_d = nc.dram_tensor("b3", [8, 1], F32, kind="ExternalInput")
    hmask_d = nc.dram_tensor("hmask", [128, NH * NH], F16, kind="ExternalInput")
    out_d = nc.dram_tensor("out", [BPC, N], F32, kind="ExternalOutput")

    with tile.TileContext(nc) as tc:
        with ExitStack() as ctx:
            const = ctx.enter_context(tc.tile_pool(name="const", bufs=1))
            inpool = ctx.enter_context(tc.tile_pool(name="inpool", bufs=2))
            work = ctx.enter_context(tc.tile_pool(name="work", bufs=2))
            acc = ctx.enter_context(tc.tile_pool(name="acc", bufs=1))
            a_ps = ctx.enter_context(tc.tile_pool(name="a_ps", bufs=2, space="PSUM"))
            cs_ps = ctx.enter_context(tc.tile_pool(name="cs_ps", bufs=2, space="PSUM"))
            mlp_ps = ctx.enter_context(tc.tile_pool(name="mlp_ps", bufs=2, space="PSUM"))

            nc.gpsimd.load_library(_mlp_lib)

            # mt first on the queue (A-proj(0) needs it); the small consts
            # are emitted after front(0) so batch 0's big loads aren't queued
            # behind them
            mt_sb = const.tile([128, NH * 128], F16)
            nc.sync.dma_start(out=mt_sb[:], in_=mt_d[:])
            # t=0 warm-up: ramp the PE p-state and force the ACT
            # function-table load before the first real batch needs them
            warm_sb = const.tile([128, 512], F16, name="warm_sb")
            nc.gpsimd.memset(warm_sb[:], 0.0)
            for wi in range(3):
                warm_ps = a_ps.tile([128, 1024], F32, space="PSUM", tag="a",
                                    name=f"warm{wi}")
                nc.tensor.matmul(out=warm_ps[:, 0:512], lhsT=warm_sb[:, 0:128],
                                 rhs=warm_sb[:], start=True, stop=True)
                if wi == 0:
                    nc.scalar.activation(out=warm_sb[0:1, 0:1],
                                         in_=warm_ps[0:1, 0:1],
                                         func=RELU, bias=0.0, scale=1.0)

            hmask_sb = const.tile([128, NH * NH], F16)
            w1e_sb = const.tile([12, 32], F16)
            b1e_sb = const.tile([32, BPC], F32)
            w2t_sb = const.tile([32, 32], F16)
            b2_sb = const.tile([64, 1], F32)
            w3blk_sb = const.tile([128, 4], F16)
            b3_sb = const.tile([8, 1], F32)

            def load_small_consts():
                nc.sync.dma_start(out=hmask_sb[:], in_=hmask_d[:])
                nc.sync.dma_start(out=w1e_sb[:], in_=w1e_d[:])
                nc.sync.dma_start(out=b1e_sb[:], in_=b1e_d[:])
                nc.sync.dma_start(out=w2t_sb[:], in_=w2t_d[:])
                nc.sync.dma_start(out=b2_sb[:], in_=b2_d[:])
                nc.sync.dma_start(out=w3blk_sb[:], in_=w3blk_d[:])
                nc.sync.dma_start(out=b3_sb[:], in_=b3_d[:])
            compat2 = []
            for i in range(2):
                t = const.tile([16, PEXT], F32, name=f"compat{i}")
                nc.gpsimd.memset(t[:], 0.0)
                compat2.append(t)

            # batched scalar-tail accumulators: tanh drains each 4-batch
            # group's L3 scores here; exp/renorm run once per [4, N] group
            tab_g = [acc.tile([4, N], F32, name="tab0", tag="tab0"),
                     acc.tile([4, N], F32, name="tab1", tag="tab1")]

            st = [dict() for _ in range(BPC)]
            mlp_state = {}

            def dma_in(b):
                s = st[b]
                pdidx_sb = inpool.tile([16, 2 * IDXW], mybir.dt.int16,
                                       tag="idx", bufs=4, name=f"pdidx{b}")
                nc.sync.dma_start(out=pdidx_sb[:], in_=pdidx_d[b])
                cv_sb = inpool.tile([128, NH], F32, tag="cv", bufs=3, name=f"cv{b}")
                nc.sync.dma_start(out=cv_sb[:], in_=cvec_d[b])
                e_bf = work.tile([128, PEXT + 2], F16, tag="e", bufs=EF_BUFS,
                                 name=f"e{b}")
                nc.sync.dma_start(out=e_bf[:], in_=hemt_d[b])
                fm1 = work.tile([128, PEXT], F16, tag="f", bufs=EF_BUFS, name=f"f{b}")
                nc.sync.dma_start(out=fm1[:], in_=fm1_d[b])
                s["e"] = e_bf
                s["f"] = fm1
                s["cv"] = cv_sb
                s["pdidx"] = pdidx_sb

            def front(b):
                s = st[b]
                e_bf, fm1, cv_sb = s["e"], s["f"], s["cv"]

                # A'_h = Mt_h^T E + c_h (fp16 matmul), drained PSUM->SBUF as
                # fp16 with the +c folded in (ACT for h<3, DVE for h=3).
                # Emission order lets DVE start products as soon as head 1's
                # ACT drain lands, instead of idling behind its own h3 drain.
                a_bf = work.tile([128, NH, PEXT], F16, tag="abf", bufs=AP_BUFS, name=f"a{b}")
                p_sb = work.tile([128, 2 * NH, PEXT], F16, tag="p", bufs=AP_BUFS, name=f"p{b}")

                def a_head(h):
                    cv = cv_sb[:, h:h + 1]
                    for pair0 in range(0, PEXT, 1024):
                        pw = min(1024, PEXT - pair0)
                        ap = a_ps.tile([128, 1024], F32, space="PSUM", tag="a",
                                       name=f"ap{b}_{h}_{pair0}")
                        for (c0, w) in [(c, min(512, pw - (c - pair0)))
                                        for c in range(pair0, pair0 + pw, 512)]:
                            nc.tensor.matmul(out=ap[:, c0 - pair0:c0 - pair0 + w],
                                             lhsT=mt_sb[:, h * 128:(h + 1) * 128],
                                             rhs=e_bf[:, c0:c0 + w], start=True, stop=True)
                        if h < 3:
                            nc.scalar.add(out=a_bf[:, h, pair0:pair0 + pw],
                                          in_=ap[:, :pw], add=cv)
                        else:
                            nc.vector.tensor_scalar_add(a_bf[:, h, pair0:pair0 + pw],
                                                        ap[:, :pw], cv)

                # products (fp16, 2x): p_sb[:, 2h, j] = A'_h[:, j] * F[:, j+1]
                #                      p_sb[:, 2h+1, j] = A'_h[:, j] * E[:, j+2]
                # (one of the 8 rides the otherwise-idle GPSIMD; the last
                # batch's are split in column halves so its colsum can begin
                # before all products finish)
                halves = ((0, PEXT),)

                def prods(h):
                    for (l0, lw) in halves:
                        if h > 0:
                            nc.vector.tensor_mul(p_sb[:, 2 * h, l0:l0 + lw],
                                                 a_bf[:, h, l0:l0 + lw],
                                                 fm1[:, l0:l0 + lw])
                        nc.vector.tensor_mul(p_sb[:, 2 * h + 1, l0:l0 + lw],
                                             a_bf[:, h, l0:l0 + lw],
                                             e_bf[:, 2 + l0:2 + l0 + lw])

                for h in range(NH):
                    a_head(h)
                nc.gpsimd.tensor_mul(p_sb[:, 0, :], a_bf[:, 0, :], fm1[:])
                for h in range(NH):
                    prods(h)
                s["p"] = p_sb

            def mid(b):
                s = st[b]
                p_sb = s["p"]
                # compat[h, pos p] = colsum(P2_h)[p] + colsum(P1_h)[p-1]
                compat_sb = compat2[b % 2]
                for (c0, w) in CHUNKS:
                    cs = cs_ps.tile([4, 512], F32, space="PSUM", tag="cs",
                                    name=f"cs{b}_{c0}")
                    for h in range(NH):
                        mk = hmask_sb[:, h * NH:(h + 1) * NH]
                        if h < NH - 1:
                            nc.tensor.matmul(out=cs[:, :w], lhsT=mk,
                                             rhs=p_sb[:, 2 * h + 1, c0:c0 + w],
                                             start=(h == 0), stop=False)
                            t1 = (cs[:, 1:w], p_sb[:, 2 * h, 0:w - 1]) if c0 == 0 else \
                                 (cs[:, :w], p_sb[:, 2 * h, c0 - 1:c0 - 1 + w])
                            nc.tensor.matmul(out=t1[0], lhsT=mk, rhs=t1[1],
                                             start=False, stop=False,
                                             skip_group_check=True)
                        else:
                            t1 = (cs[:, 1:w], p_sb[:, 2 * h, 0:w - 1]) if c0 == 0 else \
                                 (cs[:, :w], p_sb[:, 2 * h, c0 - 1:c0 - 1 + w])
                            nc.tensor.matmul(out=t1[0], lhsT=mk, rhs=t1[1],
                                             start=False, stop=False,
                                             skip_group_check=True)
                            nc.tensor.matmul(out=cs[:, :w], lhsT=mk,
                                             rhs=p_sb[:, 2 * h + 1, c0:c0 + w],
                                             start=False, stop=True)
                    nc.scalar.copy(out=compat_sb[0:4, c0:c0 + w], in_=cs[:, :w])

                pd_g = work.tile([16, 2 * NIDX], F32, tag="pdg", name=f"pdg{b}")
                nc.gpsimd.ap_gather(pd_g[:], compat_sb[:], s["pdidx"][:],
                                    channels=16, num_elems=PEXT, d=1, num_idxs=2 * NIDX)
                # pack [pickup(4); delivery(4); sig(4)] as a single 12-row
                # MLP input so L1 is one fp32r matmul per chunk
                pd12f = work.tile([12, N], F32, tag="pd12f", bufs=PD_BUFS, name=f"pd12f_{b}")
                nc.sync.dma_start(out=pd12f[0:4, :], in_=pd_g[0:4, 0:N])
                nc.sync.dma_start(out=pd12f[4:8, :], in_=pd_g[0:4, NIDX:NIDX + N])
                nc.sync.dma_start(out=pd12f[8:12, :], in_=sig_d[b])
                pd12 = work.tile([12, N], F16, tag="pd12", bufs=PD_BUFS, name=f"pd12_{b}")
                nc.vector.tensor_copy(out=pd12[:], in_=pd12f[:])
                s["pd12"] = pd12

            def mlp(b):
                s = st[b]
                pd12 = s["pd12"]
                if b % 4 == 0:
                    mlp_state["x2g"] = work.tile([128, N], F16, tag="x2g",
                                                 name=f"x2g{b // 4}")
                x2g = mlp_state["x2g"]
                p = b % 2
                if p == 0:
                    mlp_state["x2p"] = []
                x2p_t = mlp_state["x2p"]
                x1_sb = work.tile([32, N], F16, tag="x1", name=f"x1_{b}")
                for (c0, w) in MLP_CHUNKS:
                    x1p = cs_ps.tile([32, 512], F32, space="PSUM", tag="cs",
                                     name=f"x1p{b}_{c0}")
                    nc.tensor.matmul(out=x1p[:, :w], lhsT=w1e_sb[:],
                                     rhs=pd12[:, c0:c0 + w], start=True, stop=True)
                    nc.scalar.activation(out=x1_sb[:, c0:c0 + w], in_=x1p[:, :w],
                                         func=RELU, bias=b1e_sb[:, b:b + 1], scale=1.0)
                # L2 writes a [64, .] PSUM pair tile (even batch rows 0-31,
                # odd rows 32-63); one relu drains both batches at once
                for ci, (c0, w) in enumerate(MLP_CHUNKS):
                    if p == 0:
                        x2p_t.append(mlp_ps.tile([64, 512], F32, space="PSUM",
                                                 tag="m", name=f"x2p{b}_{ci}"))
                    nc.tensor.matmul(out=x2p_t[ci][32 * p:32 * p + 32, :w],
                                     lhsT=w2t_sb[:], rhs=x1_sb[:, c0:c0 + w],
                                     start=True, stop=True)
                if p == 1:
                    r0 = 64 * ((b % 4) // 2)
                    for ci, (c0, w) in enumerate(MLP_CHUNKS):
                        nc.scalar.activation(out=x2g[r0:r0 + 64, c0:c0 + w],
                                             in_=x2p_t[ci][:, :w],
                                             func=RELU, bias=b2_sb[:], scale=1.0)
                if b % 4 == 3:
                    # block-diagonal L3: one matmul yields 4 batches' scores
                    g0 = 4 * (b // 4)
                    for (c0, w) in MLP_CHUNKS:
                        l3t = cs_ps.tile([4, 512], F32, space="PSUM", tag="cs",
                                         name=f"l3t{b}_{c0}")
                        nc.tensor.matmul(out=l3t[:, :w], lhsT=w3blk_sb[:],
                                         rhs=x2g[:, c0:c0 + w], start=True, stop=True)
                        nc.scalar.activation(out=tab_g[b // 4][:, c0:c0 + w],
                                             in_=l3t[:, :w], func=TANH,
                                             bias=b3_sb[0:4, :], scale=1.0)
                    softmax_tail(b // 4)

            def softmax_tail(g):
                # probs = softmax(6 * tanh(score + b3), axis=-1) per 4-batch
                # group, emitted as soon as the group's tanh scores exist
                ex_sb = work.tile([4, N], F32, tag=f"ex{g}", bufs=1,
                                  name=f"ex{g}")
                ssum = work.tile([4, 1], F32, tag=f"ssum{g}", bufs=1,
                                 name=f"ssum{g}")
                nc.scalar.activation(out=ex_sb[:], in_=tab_g[g][:],
                                     func=EXP, bias=0.0, scale=6.0,
                                     accum_out=ssum[:])
                rcp = work.tile([4, 1], F32, tag=f"rcp{g}", bufs=1,
                                name=f"rcp{g}")
                nc.vector.reciprocal(rcp[:], ssum[:])
                probs = work.tile([4, N], F32, tag=f"probs{g}", bufs=1,
                                  name=f"probs{g}")
                nc.vector.tensor_scalar_mul(probs[:], ex_sb[:], rcp[:])
                nc.sync.dma_start(out=out_d[4 * g:4 * g + 4, :], in_=probs[:])

            # software-pipelined emission: keeps independent work in front of
            # every in-order engine queue
            dma_in(0)
            dma_in(1)
            load_small_consts()
            for i in range(BPC + 2):
                if i < BPC:
                    front(i)
                if i + 2 < BPC:
                    dma_in(i + 2)
                if 1 <= i <= BPC:
                    mid(i - 1)
                if i >= 2:
                    mlp(i - 2)

    nc.compile()
    return nc


def _decompose(perm):
    visited = np.zeros(GS, bool)
    order = []
    real = []
    for start in range(GS):
        if visited[start]:
            continue
        cyc = [start]
        visited[start] = True
        nxt = int(perm[start])
        while nxt != start:
            cyc.append(nxt)
            visited[nxt] = True
            nxt = int(perm[nxt])
        L = len(cyc)
        order.extend([cyc[-1]] + cyc + [cyc[0 % L], cyc[1 % L]])
        real.extend([False] + [True] * L + [False] * 2)
    assert len(order) <= PEXT, f"too many cycles: ext len {len(order)}"
    pad = PEXT - len(order)
    order.extend([0] * pad)
    real.extend([False] * pad)
    return np.asarray(order, np.int64), np.asarray(real, bool)


def _ext_len(perm):
    visited = np.zeros(GS, bool)
    ncyc = 0
    for start in range(GS):
        if not visited[start]:
            ncyc += 1
            visited[start] = True
            nxt = int(perm[start])
            while nxt != start:
                visited[nxt] = True
                nxt = int(perm[nxt])
    return GS + 3 * ncyc


def _idx_tile(ppos, dpos):
    idx = np.zeros(2 * NIDX, np.int16)
    idx[:ppos.shape[0]] = ppos
    idx[NIDX:NIDX + dpos.shape[0]] = dpos
    return idx.reshape(2 * IDXW, 16).T.copy()


def _f16(x):
    return np.asarray(x, np.float32).astype(np.float16)


def _host_prep(inputs):
    h_em = np.asarray(inputs["h_em"], np.float32)
    rec = np.asarray(inputs["rec"], np.int64)
    sig = np.ascontiguousarray(np.asarray(inputs["selection_sig"], np.float32))
    Wn = np.asarray(inputs["W_node"], np.float64)
    Wg = np.asarray(inputs["W_graph"], np.float64)
    WQ = np.asarray(inputs["W_Q"], np.float64)
    WK = np.asarray(inputs["W_K"], np.float64)
    w1 = np.asarray(inputs["agg_w1"], np.float64)
    b1 = np.asarray(inputs["agg_b1"], np.float64)
    w2 = np.asarray(inputs["agg_w2"], np.float32)
    b2 = np.asarray(inputs["agg_b2"], np.float32)
    w3 = np.asarray(inputs["agg_w3"], np.float32)
    b3 = np.asarray(inputs["agg_b3"], np.float32)

    Mt = np.zeros((NH, D, D), np.float64)
    C = np.zeros((NH, D, D), np.float64)
    S = np.zeros((NH, D, D), np.float64)
    for h in range(NH):
        M = WQ[h] @ WK[h].T
        Mt[h] = Wn.T @ M @ Wn
        C[h] = Wn.T @ (M + M.T) @ Wg
        S[h] = Wg.T @ M @ Wg
    mt = np.concatenate([Mt[h] for h in range(NH)], axis=1)

    g = h_em.max(axis=1).astype(np.float64)                      # (BS, D)
    cvec = np.einsum("hdf,bf->bdh", C, g).astype(np.float32)     # (BS, D, NH)
    svec = np.einsum("bd,hdf,bf->bh", g, S, g)                   # (BS, NH)
    b1_eff = (b1[None, :] + svec @ (w1[:, 0:4] + w1[:, 4:8]).T).astype(np.float32)

    w1f = w1.astype(np.float32)
    hmask = np.zeros((128, NH * NH), np.float32)
    for h in range(NH):
        hmask[:, h * NH + h] = 1.0
    w1e = np.concatenate([w1f[:, 0:4].T, w1f[:, 4:8].T, w1f[:, 8:12].T], axis=0)
    shared = {
        "mt": _f16(mt),
        "hmask": _f16(hmask),
        "w1e": _f16(w1e),
        "w2t": _f16(np.ascontiguousarray(w2.T)),
        "b2": np.tile(b2.reshape(32, 1), (2, 1)),
        "w3blk": _f16(np.kron(np.eye(4), w3.reshape(32, 1))),
        "b3": np.broadcast_to(b3.reshape(1, 1), (BPC, 1)).copy(),
    }

    in_maps = []
    for core in range(NCORES):
        b0 = core * BPC
        hemt = np.zeros((BPC, 128, PEXT + 2), np.float32)
        pdidx = np.empty((BPC, 16, 2 * IDXW), np.int16)
        for bl in range(BPC):
            order, real = _decompose(rec[b0 + bl])
            hemt[bl, :, :PEXT] = h_em[b0 + bl][order].T
            pon = np.zeros(GS, np.int64)
            pon[order[real]] = np.nonzero(real)[0]
            pdidx[bl] = _idx_tile(pon[1:N + 1], pon[N + 1:2 * N + 1])
        fm1 = np.empty((BPC, 128, PEXT), np.float32)
        fm1[:, :, :PEXT - 3] = hemt[:, :, 1:PEXT - 2] - hemt[:, :, 3:PEXT]
        fm1[:, :, PEXT - 3:] = hemt[:, :, PEXT - 3:PEXT]
        m = {
            "hemt": _f16(hemt),
            "fm1": _f16(fm1),
            "sig": sig[b0:b0 + BPC],
            "pdidx": pdidx,
            "cvec": cvec[b0:b0 + BPC],
            "b1e": np.ascontiguousarray(b1_eff[b0:b0 + BPC].T),
        }
        m.update(shared)
        in_maps.append(m)
    return in_maps


def kernel(**inputs) -> np.ndarray:
    global PEXT
    rec = np.asarray(inputs["rec"], np.int64)
    need = max(_ext_len(rec[b]) for b in range(rec.shape[0]))
    want = max(2048, -(-need // 512) * 512)
    if want != PEXT or "nc" not in _CACHE:
        PEXT = want
        _CACHE["nc"] = _build_nc()
    nc = _CACHE["nc"]
    in_maps = _host_prep(inputs)
    res = run_bass_kernel_spmd(nc, in_maps, list(range(NCORES)))
    return np.concatenate([res.results[i]["out"] for i in range(NCORES)], axis=0)
